# revision 20
# baseline (speedup 1.0000x reference)
"""MHSA block (b=8, c=256, h=w=32, nh=8) on 8 Trainium2 cores.

Sharding: pure data parallel -- one batch element per NeuronCore, no
collectives.  Per-core algorithm (X = x[b] as (C=256, L=1024)):

  QK    = Wqk @ X  (+bqk via DVE tensor_scalar add)               (512, L)
  V^T   = X^T @ WvT                                               (L, 256)
  S^T_h = K_h^T Q_h   4 heads concurrently via 4x row tiling
                      (tile_position=(32m,0), K=32 each)          (128, 1024)/jc
  P^T   = exp(scale*S^T)   (ScalarE -- the ~64us/core floor)
  O_h   = V_h^T.T @ P^T_h  4 heads concurrently via 4x col tiling
                           (tile_position=(0,32m), M=32 each;
                            PSUM accumulation over j)             (128, 512)/quad
  l_h   = ones^T @ P^T_h   4 heads col-tiled, lhsT=[1|0..0](128,32)
                           so every PSUM partition is written     row 32m = l
  O_norm = O * bcast(1/l)  (reciprocal_approx_fast + E-matmul broadcast)
  out    = (x + bproj + Wproj @ bv) + Wproj @ O_norm  (biases folded into
                                                       the fp32 residual)

All matmul operands are bf16 (PSUM accumulates fp32).  Schedule: S^T/exp/PV
run as a software pipeline (PV lags 2 slots); normalization/projection
chains are deferred into the next quad's slack.  PSUM budget (8 banks):
2x S^T (128,1024) double-buffered = 4, PV accum + denom accum x 2 quad
parities = 4.  QK / V^T / broadcast / proj borrow those slots.
"""

import sys
import os

sys.path.insert(0, "/opt/trn_rl_repo")

from contextlib import ExitStack

import numpy as np

NH, DH, C, L = 8, 32, 256, 1024
B = 8
SCALE = DH ** -0.5
N_CORES = 8
XW_W = 4384


_CACHE = {}


def _build_nc():
    import concourse.tile as tile
    from concourse import bacc, mybir

    f32 = mybir.dt.float32
    bf16 = mybir.dt.bfloat16
    Exp = mybir.ActivationFunctionType.Exp
    Identity = mybir.ActivationFunctionType.Identity

    nc = bacc.Bacc("TRN2", target_bir_lowering=False, debug=False)

    xw_d = nc.dram_tensor("xw", [128, XW_W], bf16, kind="ExternalInput").ap()
    bqkc_d = nc.dram_tensor("bqkc", [128, 6], f32, kind="ExternalInput").ap()
    out_d = nc.dram_tensor("out", [C, L], f32, kind="ExternalOutput").ap()

    with tile.TileContext(nc) as tc, ExitStack() as ctx:
        persist = ctx.enter_context(tc.tile_pool(name="persist", bufs=1))
        ptpool = ctx.enter_context(tc.tile_pool(name="pt", bufs=12))
        onpool = ctx.enter_context(tc.tile_pool(name="on", bufs=2))
        smallp = ctx.enter_context(tc.tile_pool(name="small", bufs=2))
        stps = ctx.enter_context(tc.tile_pool(name="stps", bufs=3, space="PSUM"))
        pvps = ctx.enter_context(tc.tile_pool(name="pvps", bufs=1, space="PSUM"))

        xw = persist.tile([128, XW_W], bf16, tag="xw", name="xw")
        # a-half x columns + wqk first: they alone gate the first QK matmuls
        nc.sync.dma_start(xw[:, 0:512], xw_d[:, 0:512])
        nc.sync.dma_start(xw[:, 1024:1536], xw_d[:, 1024:1536])
        nc.sync.dma_start(xw[:, 2048:2560], xw_d[:, 2048:2560])
        nc.sync.dma_start(xw[:, 2560:3072], xw_d[:, 2560:3072])
        nc.sync.dma_start(xw[:, 3072:3584], xw_d[:, 3072:3584])
        nc.sync.dma_start(xw[:, 512:1024], xw_d[:, 512:1024])
        nc.sync.dma_start(xw[:, 1536:2048], xw_d[:, 1536:2048])
        x_sb = [xw[:, 0:1024], xw[:, 1024:2048]]
        wqk_sb_w = [xw[:, 2048:2560], xw[:, 2560:3072]]
        wv_sb = [xw[:, 3072:3328], xw[:, 3328:3584]]
        wp_sb = [xw[:, 3584:3840], xw[:, 3840:4096]]
        ep_sb = xw[:, 4096:4224]
        onz_sb = xw[:, 4224:4256]
        zl_sb = xw[:, 4256:4384]  # zero lhsT for PSUM pre-clear matmuls

        bqkc_sb = persist.tile([128, 6], f32, tag="bqkc", name="bqkc")
        nc.sync.dma_start(bqkc_sb[:], bqkc_d[:])

        # warm the ACT exp table while the DMAs/QKV phase run
        warm = persist.tile([1, 8], f32, tag="warm", name="warm")
        nc.gpsimd.memset(warm[:], 0.0)
        nc.scalar.activation(warm[:], warm[:], Exp)

        # ---- QK gemm:  QK(512, L) = WqkT.T @ X; bqk added on the DVE copy.
        # Emitted in (128,512) halves on the pv/dn PSUM banks so the S^T
        # double-buffer is never disturbed and the first S^T only waits on
        # the minimal 4 matmuls. ----
        qk_sb = [persist.tile([128, L], bf16, tag=f"qk{mt}", name=f"qk{mt}")
                 for mt in range(4)]

        def qk_half(mt, half, tag, use_act=False):
            pst = stps.tile([128, L], f32, tag="st", name=f"qkps{mt}{half}")
            ps = pst[:, 0:512]
            for kt in range(2):
                nc.tensor.matmul(
                    ps,
                    lhsT=wqk_sb_w[kt][:, mt * 128:(mt + 1) * 128],
                    rhs=x_sb[kt][:, half * 512:(half + 1) * 512],
                    start=(kt == 0),
                    stop=(kt == 1),
                )
            o = qk_sb[mt][:, half * 512:(half + 1) * 512]
            if use_act:
                # ScalarE is idle before the first exp; keep the first S^T's
                # bias adds off the (slow-to-wake) DVE path
                nc.scalar.activation(o, ps, Identity, bias=bqkc_sb[:, mt:mt + 1])
            else:
                nc.vector.tensor_scalar_add(o, ps, bqkc_sb[:, mt:mt + 1])

        # ---- V^T gemm: VT(L, 256) = X.T @ WvT  (no bias; folded in residual) ----
        vt_sb = [None] * 8

        def vt_chunk(jt):
            pst = stps.tile([128, L], f32, tag="st", name="vtps")
            ps = pst
            for kt in range(2):
                nc.tensor.matmul(
                    ps[:, 0:256],
                    lhsT=x_sb[kt][:, jt * 128:(jt + 1) * 128],
                    rhs=wv_sb[kt],
                    start=(kt == 0),
                    stop=(kt == 1),
                )
            vt = persist.tile([128, 256], bf16, tag=f"vt{jt}", name=f"vt{jt}")
            nc.vector.tensor_copy(vt[:], ps[:, 0:256])
            vt_sb[jt] = vt

        # PE warm-up: ~3.5us of junk matmuls during the input DMA window so
        # HAM grants 2.4 GHz before the first QK matmul
        scratch = persist.tile([128, 512], bf16, tag="scratch", name="scratch")
        nc.gpsimd.memset(scratch[:], 0.0)
        wps = pvps.tile([128, 512], f32, tag="pv", name="warmps")
        for i in range(8):
            nc.tensor.matmul(wps[:], lhsT=scratch[:, 0:128], rhs=scratch[:],
                             start=(i == 0), stop=(i == 7))
        nc.vector.tensor_copy(scratch[:, 0:128], wps[:, 0:128])

        qk_half(0, 0, "st", use_act=True)
        qk_half(2, 0, "st", use_act=True)
        nc.sync.dma_start(xw[:, 3584:XW_W], xw_d[:, 3584:XW_W])

        # residual xf = bf16(x) + (bproj + Wproj@bv) computed on-chip; saves
        # the 1MB fp32 DMA that would otherwise fight x/w for startup HBM BW
        xf_sb = [persist.tile([128, L], f32, tag=f"xf{t}", name=f"xf{t}")
                 for t in range(2)]

        def xf_make():
            for t in range(2):
                nc.vector.tensor_scalar_add(xf_sb[t][:], x_sb[t], bqkc_sb[:, 4 + t:5 + t])

        acc = [persist.tile([128, L], f32, tag=f"acc{t}", name=f"acc{t}") for t in range(2)]

        deferred = []
        on_holder = {}

        def make_chain_a1(tg, ih, qi, pv, dn):
            def chain_a1():
                # rows 32m of dn hold l_h; other rows are exact zeros, so the
                # E' broadcast matmul (E'[32k, 32k+d] = 1) replicates l_h to
                # its head's 32 rows without picking up garbage.  a1 fully
                # evacuates BOTH accumulator banks to SBUF so the next quad's
                # accumulators can claim them after two DVE copies instead of
                # after the whole recip/mul chain.
                df = smallp.tile([128, 512], bf16, tag="df", name="df")
                nc.vector.tensor_copy(df[:], dn[:])
                if qi < 3:
                    pvs = smallp.tile([128, 512], bf16, tag="pvs", name="pvs")
                    nc.vector.tensor_copy(pvs[:], pv[:])
                else:
                    pvs = pv  # no next quad waits on the bank; mul reads PSUM
                on_holder[("df", qi)] = (df, pvs)

            return chain_a1

        def make_chain_a2(tg, ih, qi, pv, dn):
            parts = ((0, 256), (256, 256)) if qi == 3 else ((0, 512),)

            def chain_a2():
                df, pvs = on_holder.pop(("df", qi))
                on = onpool.tile([128, 512], bf16, tag="on", name="on")
                lbt = stps.tile([128, L], f32, tag="st", name="lb")
                lb = lbt[:, 0:512]
                for o, w in parts:
                    nc.tensor.matmul(lb[:, o:o + w], lhsT=ep_sb, rhs=df[:, o:o + w],
                                     start=True, stop=True)
                    rps = smallp.tile([128, 512], f32, tag="rps", name="rps")
                    nc.vector.reciprocal_approx_fast(rps[:, 0:w], lb[:, o:o + w])
                    nc.vector.tensor_mul(on[:, o:o + w], pvs[:, o:o + w], rps[:, 0:w])
                if qi == 3:
                    # keep HAM warm through the final DVE chain so the
                    # projection matmuls run at 2.4 GHz
                    for i in range(6):
                        nc.tensor.matmul(lbt[:, 512:768], lhsT=zl_sb, rhs=scratch[:, 0:256],
                                         start=(i == 0), stop=(i == 5))
                    nc.vector.tensor_copy(scratch[:, 128:256], lbt[:, 512:640])
                on_holder[qi] = on

            return chain_a2

        def make_chain_b(tg, ih, qi):
            cols = slice(ih * 512, (ih + 1) * 512)

            def chain_b():
                on = on_holder.pop(qi)
                pj = stps.tile([128, L], f32, tag="st", name="pj")
                for mt in range(2):
                    nc.tensor.matmul(
                        pj[:, mt * 512:(mt + 1) * 512],
                        lhsT=wp_sb[tg][:, mt * 128:(mt + 1) * 128],
                        rhs=on[:],
                        start=True,
                        stop=True,
                    )
                for mt in range(2):
                    pjv = pj[:, mt * 512:(mt + 1) * 512]
                    if tg == 0:
                        nc.vector.tensor_add(acc[mt][:, cols], xf_sb[mt][:, cols], pjv)
                    else:
                        nc.vector.tensor_add(acc[mt][:, cols], acc[mt][:, cols], pjv)
                        nc.sync.dma_start(out_d[mt * 128:(mt + 1) * 128, cols], acc[mt][:, cols])

            return chain_b

        def make_quad(qi, tg, ih):
            cols = slice(ih * 512, (ih + 1) * 512)
            state = {}

            def st_fn(jc):
                for h in iter_hooks.pop((tg, ih, jc), []):
                    h()
                qt = qk_sb[tg]
                kt_ = qk_sb[2 + tg]
                sts = [
                    stps.tile([128, L], f32, tag="st", name="stA"),
                    stps.tile([128, L], f32, tag="st", name="stB"),
                ]
                for m in range(4):
                    o = 32 * m
                    nc.tensor.matmul(
                        sts[m // 2][:, (m % 2) * 512:(m % 2) * 512 + 512],
                        lhsT=kt_[o:o + 32, jc * 128:(jc + 1) * 128],
                        rhs=qt[o:o + 32, cols],
                        start=True,
                        stop=True,
                        tile_position=(o, 0),
                    )
                pts = []
                for half in range(2):
                    pt = ptpool.tile([128, L], bf16, tag="pt", name="pt")
                    nc.scalar.activation(pt[:], sts[half][:], Exp, scale=SCALE)
                    pts.append(pt)
                state[jc] = pts

            def pv_fn(jc):
                if jc == 0:
                    # Pre-zero both accumulators with one full-array matmul
                    # each: concurrent col-tiled matmuls must not carry
                    # start=True (the bank-wide has_written clear races
                    # against sibling tiles writing the same bank).
                    state["pv"] = pvps.tile([128, 512], f32, tag="pv", name="pvacc")
                    state["dn"] = pvps.tile([128, 512], f32, tag="dn", name="dnacc")
                    for z in ("pv", "dn"):
                        nc.tensor.matmul(
                            state[z][:],
                            lhsT=zl_sb,
                            rhs=x_sb[0][:, 0:512],
                            start=True,
                            stop=True,
                        )
                pts = state.pop(jc)
                pv, dn = state["pv"], state["dn"]
                for m in range(4):
                    nc.tensor.matmul(
                        pv[32 * m:32 * m + 32, :],
                        lhsT=vt_sb[jc][:, (4 * tg + m) * 32:(4 * tg + m) * 32 + 32],
                        rhs=pts[m // 2][:, (m % 2) * 512:(m % 2) * 512 + 512],
                        start=False,
                        stop=(jc == 7),
                        tile_position=(0, 32 * m),
                    )
                for m in range(4):
                    nc.tensor.matmul(
                        dn[32 * m:32 * m + 32, :],
                        lhsT=onz_sb,
                        rhs=pts[m // 2][:, (m % 2) * 512:(m % 2) * 512 + 512],
                        start=False,
                        stop=(jc == 7),
                        tile_position=(0, 32 * m),
                    )
                if jc == 7:
                    deferred.append(make_chain_a1(tg, ih, qi, pv, dn))
                    deferred.append(make_chain_a2(tg, ih, qi, pv, dn))
                    deferred.append(make_chain_b(tg, ih, qi))

            return st_fn, pv_fn

        quads = [(0, 0), (0, 1), (1, 0), (1, 1)]
        iter_hooks = {
            (0, 0, 1): [lambda: [vt_chunk(j) for j in range(2)]],
            (0, 0, 2): [lambda: qk_half(2, 1, "st"),
                        lambda: [vt_chunk(j) for j in range(2, 4)]],
            (0, 0, 3): [xf_make, lambda: qk_half(0, 1, "st"),
                        lambda: [vt_chunk(j) for j in range(4, 6)]],
            (0, 0, 4): [lambda: [vt_chunk(j) for j in range(6, 8)]],
            (0, 1, 1): [lambda: qk_half(1, 0, "st")],
            (0, 1, 4): [lambda: qk_half(1, 1, "st")],
            (0, 1, 6): [lambda: qk_half(3, 0, "st")],
            (0, 1, 7): [lambda: qk_half(3, 1, "st")],
        }
        # Explicit slot schedule.  Slot s = 8q + r emits S^T(q, r); the PV
        # groups lag 2-3 slots; the norm/proj chain of quad q-1 is split
        # across r = 2 (DVE copy), r = 3 (broadcast+recip+mul, emitted just
        # before quad q's accumulator allocs so the pv/dn banks hand over
        # without a cycle), and r = 5 (projection).
        fns = [make_quad(qi, tg, ih) for qi, (tg, ih) in enumerate(quads)]
        pv_slots = {}
        for q in range(4):
            for j in range(8):
                pv_slots.setdefault(8 * q + j + 2, []).append((q, j))

        def emit_pvs(s):
            for pq, pj in pv_slots.get(s, []):
                fns[pq][1](pj)

        for s in range(32):
            q, r = divmod(s, 8)
            fns[q][0](r)
            # chain a1 frees the pv/dn banks this slot's pv(q,0) allocs claim
            if r in (2, 3, 5) and deferred:
                deferred.pop(0)()
            emit_pvs(s)
        for s in (32, 33):
            emit_pvs(s)
        while deferred:
            deferred.pop(0)()

    nc.compile()
    return nc


def _get_nc():
    if "nc" not in _CACHE:
        _CACHE["nc"] = _build_nc()
    return _CACHE["nc"]


def _pack_weights(w_qkv, b_qkv, w_proj, b_proj):
    w_qkv = np.asarray(w_qkv, dtype=np.float32)
    b_qkv = np.asarray(b_qkv, dtype=np.float32)
    w_proj = np.asarray(w_proj, dtype=np.float32)
    b_proj = np.asarray(b_proj, dtype=np.float32)

    wqkT = np.ascontiguousarray(w_qkv[:512].T)                  # (256, 512)
    resid_bias = b_proj + w_proj @ b_qkv[512:768]
    bqkc = np.zeros((128, 6), dtype=np.float32)
    bqkc[:, 0:4] = b_qkv[:512].reshape(4, 128).T
    bqkc[:, 4] = resid_bias[0:128]
    bqkc[:, 5] = resid_bias[128:256]
    wvT = np.ascontiguousarray(w_qkv[512:768].T)                # (256, 256)
    wpT = np.ascontiguousarray(w_proj.T)                        # (256, 256)

    wext = np.zeros((128, XW_W - 2048), dtype=np.float32)
    wext[:, 0:512] = wqkT[0:128]
    wext[:, 512:1024] = wqkT[128:256]
    wext[:, 1024:1280] = wvT[0:128]
    wext[:, 1280:1536] = wvT[128:256]
    wext[:, 1536:1792] = wpT[0:128]
    wext[:, 1792:2048] = wpT[128:256]
    # E' broadcast matrix (128,128): E'[32k, 32k+d] = 1 replicates the
    # denominator at row 32k to rows 32k..32k+31 of its head
    for k in range(4):
        wext[32 * k, 2048 + 32 * k:2048 + 32 * k + 32] = 1.0
    # ones|zeros (128,32) lhsT for the denominator matmuls
    wext[:, 2176] = 1.0
    # the residual carries x + bproj + Wproj @ bv (the V-bias contribution:
    # O_norm = O/l + bv, and Wproj @ bv is column-constant); it is built
    # on-chip from bf16(x) + bqkc cols 4:6
    return wext, bqkc


def _bf16(a):
    import ml_dtypes

    return np.asarray(a).astype(ml_dtypes.bfloat16)


def _install_ntff_hook_module():
    """bass_utils wants antenv.axon_hooks for trace=True under axon; this
    image's antenv lacks it.  Inject an equivalent module into sys.modules."""
    if "antenv.axon_hooks" in sys.modules:
        return
    try:
        import antenv.axon_hooks  # noqa: F401

        return
    except ImportError:
        pass
    import contextlib
    import ctypes
    import types

    mod = types.ModuleType("antenv.axon_hooks")
    state = {"hook": None, "inited": False}

    def _default_hook():
        so_path = "/opt/axon/libaxon_pjrt.so"
        if not os.path.exists(so_path):
            return None
        lib = ctypes.CDLL(so_path)
        if not hasattr(lib, "axon_start_nrt_profile"):
            return None
        lib.axon_start_nrt_profile.argtypes = [
            ctypes.POINTER(ctypes.c_int64),
            ctypes.c_size_t,
        ]
        lib.axon_start_nrt_profile.restype = ctypes.c_int64
        lib.axon_stop_nrt_profile.argtypes = [ctypes.c_char_p]
        lib.axon_stop_nrt_profile.restype = ctypes.c_int64

        @contextlib.contextmanager
        def _hook(output_dir, device_ids):
            import jax

            jax.devices()
            if device_ids:
                ids = (ctypes.c_int64 * len(device_ids))(*device_ids)
                rc = lib.axon_start_nrt_profile(ids, len(device_ids))
            else:
                rc = lib.axon_start_nrt_profile(None, 0)
            if rc != 0:
                raise RuntimeError(f"axon_start_nrt_profile rc={rc}")
            try:
                yield
            finally:
                n = lib.axon_stop_nrt_profile(str(output_dir).encode())
                if n < 0:
                    raise RuntimeError(f"axon_stop_nrt_profile rc={n}")
                print(f"profile: {n} file(s) written to {output_dir}")

        return _hook

    def set_axon_ntff_profile_hook(hook):
        state["hook"] = hook
        state["inited"] = True

    def get_axon_ntff_profile_hook():
        if not state["inited"]:
            state["hook"] = _default_hook()
            state["inited"] = True
        return state["hook"]

    mod.set_axon_ntff_profile_hook = set_axon_ntff_profile_hook
    mod.get_axon_ntff_profile_hook = get_axon_ntff_profile_hook
    sys.modules["antenv.axon_hooks"] = mod


def _prepare_in_maps(x, w_qkv, b_qkv, w_proj, b_proj):
    x = np.asarray(x, dtype=np.float32)
    b, c, h, w = x.shape
    assert (b, c, h, w) == (B, C, 32, 32)

    wext, bqkc = _pack_weights(w_qkv, b_qkv, w_proj, b_proj)
    wext_bf = _bf16(wext)
    bqkc = np.ascontiguousarray(bqkc, dtype=np.float32)

    in_maps = []
    for core in range(N_CORES):
        xm = np.ascontiguousarray(x[core].reshape(C, L))
        xw = np.empty((128, XW_W), dtype=wext_bf.dtype)
        xw[:, 0:1024] = _bf16(xm[0:128])
        xw[:, 1024:2048] = _bf16(xm[128:256])
        xw[:, 2048:XW_W] = wext_bf
        m = {"xw": xw, "bqkc": bqkc}
        in_maps.append(m)
    return in_maps


def kernel(x, w_qkv, b_qkv, w_proj, b_proj, _trace=False, _trace_kwargs=None):
    if _trace:
        _install_ntff_hook_module()
    from concourse.bass_utils import run_bass_kernel_spmd

    in_maps = _prepare_in_maps(x, w_qkv, b_qkv, w_proj, b_proj)
    nc = _get_nc()

    res = run_bass_kernel_spmd(
        nc,
        in_maps,
        list(range(N_CORES)),
        trace=_trace,
        **(_trace_kwargs or {}),
    )
    out = np.stack([res.results[core]["out"] for core in range(N_CORES)])
    if _trace:
        _CACHE["last_result"] = res
    return out.reshape(B, C, 32, 32)


# revision 21
# speedup vs baseline: 1.0221x; 1.0221x over previous
"""MHSA block (b=8, c=256, h=w=32, nh=8) on 8 Trainium2 cores.

Sharding: pure data parallel -- one batch element per NeuronCore, no
collectives.  Per-core algorithm (X = x[b] as (C=256, L=1024)):

  QK    = Wqk @ X  (+bqk via DVE tensor_scalar add)               (512, L)
  V^T   = X^T @ WvT                                               (L, 256)
  S^T_h = K_h^T Q_h   4 heads concurrently via 4x row tiling
                      (tile_position=(32m,0), K=32 each)          (128, 1024)/jc
  P^T   = exp(scale*S^T)   (ScalarE -- the ~64us/core floor)
  O_h   = V_h^T.T @ P^T_h  4 heads concurrently via 4x col tiling
                           (tile_position=(0,32m), M=32 each;
                            PSUM accumulation over j)             (128, 512)/quad
  l_h   = ones^T @ P^T_h   4 heads col-tiled, lhsT=[1|0..0](128,32)
                           so every PSUM partition is written     row 32m = l
  O_norm = O * bcast(1/l)  (reciprocal_approx_fast + E-matmul broadcast)
  out    = (x + bproj + Wproj @ bv) + Wproj @ O_norm  (biases folded into
                                                       the fp32 residual)

All matmul operands are bf16 (PSUM accumulates fp32).  Schedule: S^T/exp/PV
run as a software pipeline (PV lags 2 slots); normalization/projection
chains are deferred into the next quad's slack.  PSUM budget (8 banks):
2x S^T (128,1024) double-buffered = 4, PV accum + denom accum x 2 quad
parities = 4.  QK / V^T / broadcast / proj borrow those slots.
"""

import sys
import os

sys.path.insert(0, "/opt/trn_rl_repo")

from contextlib import ExitStack

import numpy as np

NH, DH, C, L = 8, 32, 256, 1024
B = 8
SCALE = DH ** -0.5
N_CORES = 8
XW_W = 4384


_CACHE = {}


def _build_nc():
    import concourse.tile as tile
    from concourse import bacc, mybir

    f32 = mybir.dt.float32
    bf16 = mybir.dt.bfloat16
    Exp = mybir.ActivationFunctionType.Exp
    Identity = mybir.ActivationFunctionType.Identity

    nc = bacc.Bacc("TRN2", target_bir_lowering=False, debug=False)

    xw_d = nc.dram_tensor("xw", [128, XW_W], bf16, kind="ExternalInput").ap()
    bqkc_d = nc.dram_tensor("bqkc", [128, 6], f32, kind="ExternalInput").ap()
    out_d = nc.dram_tensor("out", [C, L], f32, kind="ExternalOutput").ap()

    with tile.TileContext(nc) as tc, ExitStack() as ctx:
        persist = ctx.enter_context(tc.tile_pool(name="persist", bufs=1))
        ptpool = ctx.enter_context(tc.tile_pool(name="pt", bufs=12))
        onpool = ctx.enter_context(tc.tile_pool(name="on", bufs=2))
        smallp = ctx.enter_context(tc.tile_pool(name="small", bufs=2))
        stps = ctx.enter_context(tc.tile_pool(name="stps", bufs=3, space="PSUM"))
        pvps = ctx.enter_context(tc.tile_pool(name="pvps", bufs=1, space="PSUM"))

        xw = persist.tile([128, XW_W], bf16, tag="xw", name="xw")
        # a-half x columns + wqk first: they alone gate the first QK matmuls
        nc.sync.dma_start(xw[:, 0:512], xw_d[:, 0:512])
        nc.sync.dma_start(xw[:, 1024:1536], xw_d[:, 1024:1536])
        nc.sync.dma_start(xw[:, 2048:2560], xw_d[:, 2048:2560])
        nc.sync.dma_start(xw[:, 2560:3072], xw_d[:, 2560:3072])
        nc.sync.dma_start(xw[:, 3072:3584], xw_d[:, 3072:3584])
        nc.sync.dma_start(xw[:, 512:1024], xw_d[:, 512:1024])
        nc.sync.dma_start(xw[:, 1536:2048], xw_d[:, 1536:2048])
        x_sb = [xw[:, 0:1024], xw[:, 1024:2048]]
        wqk_sb_w = [xw[:, 2048:2560], xw[:, 2560:3072]]
        wv_sb = [xw[:, 3072:3328], xw[:, 3328:3584]]
        wp_sb = [xw[:, 3584:3840], xw[:, 3840:4096]]
        ep_sb = xw[:, 4096:4224]
        onz_sb = xw[:, 4224:4256]
        zl_sb = xw[:, 4256:4384]  # zero lhsT for PSUM pre-clear matmuls

        bqkc_sb = persist.tile([128, 6], f32, tag="bqkc", name="bqkc")
        nc.sync.dma_start(bqkc_sb[:], bqkc_d[:])

        # warm the ACT exp table while the DMAs/QKV phase run
        warm = persist.tile([1, 8], f32, tag="warm", name="warm")
        nc.gpsimd.memset(warm[:], 0.0)
        nc.scalar.activation(warm[:], warm[:], Exp)

        # ---- QK gemm:  QK(512, L) = WqkT.T @ X; bqk added on the DVE copy.
        # Emitted in (128,512) halves on the pv/dn PSUM banks so the S^T
        # double-buffer is never disturbed and the first S^T only waits on
        # the minimal 4 matmuls. ----
        qk_sb = [persist.tile([128, L], bf16, tag=f"qk{mt}", name=f"qk{mt}")
                 for mt in range(4)]

        def qk_half(mt, half, tag, use_act=False):
            pst = stps.tile([128, L], f32, tag="st", name=f"qkps{mt}{half}")
            ps = pst[:, 0:512]
            for kt in range(2):
                nc.tensor.matmul(
                    ps,
                    lhsT=wqk_sb_w[kt][:, mt * 128:(mt + 1) * 128],
                    rhs=x_sb[kt][:, half * 512:(half + 1) * 512],
                    start=(kt == 0),
                    stop=(kt == 1),
                )
            o = qk_sb[mt][:, half * 512:(half + 1) * 512]
            if use_act:
                # ScalarE is idle before the first exp; keep the first S^T's
                # bias adds off the (slow-to-wake) DVE path
                nc.scalar.activation(o, ps, Identity, bias=bqkc_sb[:, mt:mt + 1])
            else:
                nc.vector.tensor_scalar_add(o, ps, bqkc_sb[:, mt:mt + 1])

        # ---- V^T gemm: VT(L, 256) = X.T @ WvT  (no bias; folded in residual) ----
        vt_sb = [None] * 8

        def vt_chunk(jt):
            pst = stps.tile([128, L], f32, tag="st", name="vtps")
            ps = pst
            for kt in range(2):
                nc.tensor.matmul(
                    ps[:, 0:256],
                    lhsT=x_sb[kt][:, jt * 128:(jt + 1) * 128],
                    rhs=wv_sb[kt],
                    start=(kt == 0),
                    stop=(kt == 1),
                )
            vt = persist.tile([128, 256], bf16, tag=f"vt{jt}", name=f"vt{jt}")
            nc.vector.tensor_copy(vt[:], ps[:, 0:256])
            vt_sb[jt] = vt

        # PE warm-up: ~3.5us of junk matmuls during the input DMA window so
        # HAM grants 2.4 GHz before the first QK matmul
        scratch = persist.tile([128, 512], bf16, tag="scratch", name="scratch")
        nc.gpsimd.memset(scratch[:], 0.0)
        wps = pvps.tile([128, 512], f32, tag="pv", name="warmps")
        for i in range(8):
            nc.tensor.matmul(wps[:], lhsT=scratch[:, 0:128], rhs=scratch[:],
                             start=(i == 0), stop=(i == 7))
        nc.vector.tensor_copy(scratch[:, 0:128], wps[:, 0:128])

        qk_half(0, 0, "st", use_act=True)
        qk_half(2, 0, "st", use_act=True)
        nc.sync.dma_start(xw[:, 3584:XW_W], xw_d[:, 3584:XW_W])

        # residual xf = bf16(x) + (bproj + Wproj@bv) computed on-chip; saves
        # the 1MB fp32 DMA that would otherwise fight x/w for startup HBM BW
        xf_sb = [persist.tile([128, L], f32, tag=f"xf{t}", name=f"xf{t}")
                 for t in range(2)]

        def xf_make():
            for t in range(2):
                nc.vector.tensor_scalar_add(xf_sb[t][:], x_sb[t], bqkc_sb[:, 4 + t:5 + t])

        acc = [persist.tile([128, L], f32, tag=f"acc{t}", name=f"acc{t}") for t in range(2)]

        deferred = []
        on_holder = {}

        def make_chain_a1(tg, ih, qi, pv, dn):
            def chain_a1():
                # rows 32m of dn hold l_h; other rows are exact zeros, so the
                # E' broadcast matmul (E'[32k, 32k+d] = 1) replicates l_h to
                # its head's 32 rows without picking up garbage.  a1 fully
                # evacuates BOTH accumulator banks to SBUF so the next quad's
                # accumulators can claim them after two DVE copies instead of
                # after the whole recip/mul chain.
                df = smallp.tile([128, 512], bf16, tag="df", name="df")
                nc.vector.tensor_copy(df[:], dn[:])
                if qi < 3:
                    pvs = smallp.tile([128, 512], bf16, tag="pvs", name="pvs")
                    nc.vector.tensor_copy(pvs[:], pv[:])
                else:
                    pvs = pv  # no next quad waits on the bank; mul reads PSUM
                on_holder[("df", qi)] = (df, pvs)

            return chain_a1

        def make_chain_a2(tg, ih, qi, pv, dn):
            parts = ((0, 256), (256, 256)) if qi == 3 else ((0, 512),)

            def chain_a2():
                df, pvs = on_holder.pop(("df", qi))
                on = onpool.tile([128, 512], bf16, tag="on", name="on")
                lbt = stps.tile([128, L], f32, tag="st", name="lb")
                lb = lbt[:, 0:512]
                for o, w in parts:
                    nc.tensor.matmul(lb[:, o:o + w], lhsT=ep_sb, rhs=df[:, o:o + w],
                                     start=True, stop=True)
                    rps = smallp.tile([128, 512], f32, tag="rps", name="rps")
                    nc.vector.reciprocal_approx_fast(rps[:, 0:w], lb[:, o:o + w])
                    nc.vector.tensor_mul(on[:, o:o + w], pvs[:, o:o + w], rps[:, 0:w])
                on_holder[qi] = on

            return chain_a2

        def make_chain_b(tg, ih, qi):
            cols = slice(ih * 512, (ih + 1) * 512)

            def chain_b():
                on = on_holder.pop(qi)
                pj = stps.tile([128, L], f32, tag="st", name="pj")
                for mt in range(2):
                    nc.tensor.matmul(
                        pj[:, mt * 512:(mt + 1) * 512],
                        lhsT=wp_sb[tg][:, mt * 128:(mt + 1) * 128],
                        rhs=on[:],
                        start=True,
                        stop=True,
                    )
                for mt in range(2):
                    pjv = pj[:, mt * 512:(mt + 1) * 512]
                    if tg == 0:
                        nc.vector.tensor_add(acc[mt][:, cols], xf_sb[mt][:, cols], pjv)
                    else:
                        nc.vector.tensor_add(acc[mt][:, cols], acc[mt][:, cols], pjv)
                        nc.sync.dma_start(out_d[mt * 128:(mt + 1) * 128, cols], acc[mt][:, cols])

            return chain_b

        def make_quad(qi, tg, ih):
            cols = slice(ih * 512, (ih + 1) * 512)
            state = {}

            def st_fn(jc):
                for h in iter_hooks.pop((tg, ih, jc), []):
                    h()
                qt = qk_sb[tg]
                kt_ = qk_sb[2 + tg]
                sts = [
                    stps.tile([128, L], f32, tag="st", name="stA"),
                    stps.tile([128, L], f32, tag="st", name="stB"),
                ]
                for m in range(4):
                    o = 32 * m
                    nc.tensor.matmul(
                        sts[m // 2][:, (m % 2) * 512:(m % 2) * 512 + 512],
                        lhsT=kt_[o:o + 32, jc * 128:(jc + 1) * 128],
                        rhs=qt[o:o + 32, cols],
                        start=True,
                        stop=True,
                        tile_position=(o, 0),
                    )
                pts = []
                for half in range(2):
                    pt = ptpool.tile([128, L], bf16, tag="pt", name="pt")
                    nc.scalar.activation(pt[:], sts[half][:], Exp, scale=SCALE)
                    pts.append(pt)
                state[jc] = pts

            def pv_fn(jc):
                if jc == 0:
                    # Pre-zero both accumulators with one full-array matmul
                    # each: concurrent col-tiled matmuls must not carry
                    # start=True (the bank-wide has_written clear races
                    # against sibling tiles writing the same bank).
                    state["pv"] = pvps.tile([128, 512], f32, tag="pv", name="pvacc")
                    state["dn"] = pvps.tile([128, 512], f32, tag="dn", name="dnacc")
                    for z in ("pv", "dn"):
                        nc.tensor.matmul(
                            state[z][:],
                            lhsT=zl_sb,
                            rhs=x_sb[0][:, 0:512],
                            start=True,
                            stop=True,
                        )
                pts = state.pop(jc)
                pv, dn = state["pv"], state["dn"]
                for m in range(4):
                    nc.tensor.matmul(
                        pv[32 * m:32 * m + 32, :],
                        lhsT=vt_sb[jc][:, (4 * tg + m) * 32:(4 * tg + m) * 32 + 32],
                        rhs=pts[m // 2][:, (m % 2) * 512:(m % 2) * 512 + 512],
                        start=False,
                        stop=(jc == 7),
                        tile_position=(0, 32 * m),
                    )
                for m in range(4):
                    nc.tensor.matmul(
                        dn[32 * m:32 * m + 32, :],
                        lhsT=onz_sb,
                        rhs=pts[m // 2][:, (m % 2) * 512:(m % 2) * 512 + 512],
                        start=False,
                        stop=(jc == 7),
                        tile_position=(0, 32 * m),
                    )
                if jc == 7:
                    deferred.append(make_chain_a1(tg, ih, qi, pv, dn))
                    deferred.append(make_chain_a2(tg, ih, qi, pv, dn))
                    deferred.append(make_chain_b(tg, ih, qi))

            return st_fn, pv_fn

        quads = [(0, 0), (0, 1), (1, 0), (1, 1)]
        iter_hooks = {
            (0, 0, 1): [lambda: qk_half(2, 1, "st"),
                        lambda: [vt_chunk(j) for j in range(2)]],
            (0, 0, 3): [xf_make, lambda: [vt_chunk(j) for j in range(2, 4)]],
            (0, 0, 4): [lambda: [vt_chunk(j) for j in range(4, 6)]],
            (0, 0, 5): [lambda: qk_half(0, 1, "st")],
            (0, 0, 6): [lambda: [vt_chunk(j) for j in range(6, 8)]],
            (0, 1, 1): [lambda: qk_half(1, 0, "st")],
            (0, 1, 4): [lambda: qk_half(1, 1, "st")],
            (0, 1, 6): [lambda: qk_half(3, 0, "st")],
            (0, 1, 7): [lambda: qk_half(3, 1, "st")],
        }
        # Explicit slot schedule.  Slot s = 8q + r emits S^T(q, r); the PV
        # groups lag 2-3 slots; the norm/proj chain of quad q-1 is split
        # across r = 2 (DVE copy), r = 3 (broadcast+recip+mul, emitted just
        # before quad q's accumulator allocs so the pv/dn banks hand over
        # without a cycle), and r = 5 (projection).
        fns = [make_quad(qi, tg, ih) for qi, (tg, ih) in enumerate(quads)]
        pv_slots = {}
        for q in range(4):
            for j in range(8):
                pv_slots.setdefault(8 * q + j + 2, []).append((q, j))

        def emit_pvs(s):
            for pq, pj in pv_slots.get(s, []):
                fns[pq][1](pj)

        for s in range(32):
            q, r = divmod(s, 8)
            fns[q][0](r)
            # chain a1 frees the pv/dn banks this slot's pv(q,0) allocs claim
            if r in (2, 3, 5) and deferred:
                deferred.pop(0)()
            emit_pvs(s)
        for s in (32, 33):
            emit_pvs(s)
        while deferred:
            deferred.pop(0)()

    nc.compile()
    return nc


def _get_nc():
    if "nc" not in _CACHE:
        _CACHE["nc"] = _build_nc()
    return _CACHE["nc"]


def _pack_weights(w_qkv, b_qkv, w_proj, b_proj):
    w_qkv = np.asarray(w_qkv, dtype=np.float32)
    b_qkv = np.asarray(b_qkv, dtype=np.float32)
    w_proj = np.asarray(w_proj, dtype=np.float32)
    b_proj = np.asarray(b_proj, dtype=np.float32)

    wqkT = np.ascontiguousarray(w_qkv[:512].T)                  # (256, 512)
    resid_bias = b_proj + w_proj @ b_qkv[512:768]
    bqkc = np.zeros((128, 6), dtype=np.float32)
    bqkc[:, 0:4] = b_qkv[:512].reshape(4, 128).T
    bqkc[:, 4] = resid_bias[0:128]
    bqkc[:, 5] = resid_bias[128:256]
    wvT = np.ascontiguousarray(w_qkv[512:768].T)                # (256, 256)
    wpT = np.ascontiguousarray(w_proj.T)                        # (256, 256)

    wext = np.zeros((128, XW_W - 2048), dtype=np.float32)
    wext[:, 0:512] = wqkT[0:128]
    wext[:, 512:1024] = wqkT[128:256]
    wext[:, 1024:1280] = wvT[0:128]
    wext[:, 1280:1536] = wvT[128:256]
    wext[:, 1536:1792] = wpT[0:128]
    wext[:, 1792:2048] = wpT[128:256]
    # E' broadcast matrix (128,128): E'[32k, 32k+d] = 1 replicates the
    # denominator at row 32k to rows 32k..32k+31 of its head
    for k in range(4):
        wext[32 * k, 2048 + 32 * k:2048 + 32 * k + 32] = 1.0
    # ones|zeros (128,32) lhsT for the denominator matmuls
    wext[:, 2176] = 1.0
    # the residual carries x + bproj + Wproj @ bv (the V-bias contribution:
    # O_norm = O/l + bv, and Wproj @ bv is column-constant); it is built
    # on-chip from bf16(x) + bqkc cols 4:6
    return wext, bqkc


def _bf16(a):
    import ml_dtypes

    return np.asarray(a).astype(ml_dtypes.bfloat16)


def _install_ntff_hook_module():
    """bass_utils wants antenv.axon_hooks for trace=True under axon; this
    image's antenv lacks it.  Inject an equivalent module into sys.modules."""
    if "antenv.axon_hooks" in sys.modules:
        return
    try:
        import antenv.axon_hooks  # noqa: F401

        return
    except ImportError:
        pass
    import contextlib
    import ctypes
    import types

    mod = types.ModuleType("antenv.axon_hooks")
    state = {"hook": None, "inited": False}

    def _default_hook():
        so_path = "/opt/axon/libaxon_pjrt.so"
        if not os.path.exists(so_path):
            return None
        lib = ctypes.CDLL(so_path)
        if not hasattr(lib, "axon_start_nrt_profile"):
            return None
        lib.axon_start_nrt_profile.argtypes = [
            ctypes.POINTER(ctypes.c_int64),
            ctypes.c_size_t,
        ]
        lib.axon_start_nrt_profile.restype = ctypes.c_int64
        lib.axon_stop_nrt_profile.argtypes = [ctypes.c_char_p]
        lib.axon_stop_nrt_profile.restype = ctypes.c_int64

        @contextlib.contextmanager
        def _hook(output_dir, device_ids):
            import jax

            jax.devices()
            if device_ids:
                ids = (ctypes.c_int64 * len(device_ids))(*device_ids)
                rc = lib.axon_start_nrt_profile(ids, len(device_ids))
            else:
                rc = lib.axon_start_nrt_profile(None, 0)
            if rc != 0:
                raise RuntimeError(f"axon_start_nrt_profile rc={rc}")
            try:
                yield
            finally:
                n = lib.axon_stop_nrt_profile(str(output_dir).encode())
                if n < 0:
                    raise RuntimeError(f"axon_stop_nrt_profile rc={n}")
                print(f"profile: {n} file(s) written to {output_dir}")

        return _hook

    def set_axon_ntff_profile_hook(hook):
        state["hook"] = hook
        state["inited"] = True

    def get_axon_ntff_profile_hook():
        if not state["inited"]:
            state["hook"] = _default_hook()
            state["inited"] = True
        return state["hook"]

    mod.set_axon_ntff_profile_hook = set_axon_ntff_profile_hook
    mod.get_axon_ntff_profile_hook = get_axon_ntff_profile_hook
    sys.modules["antenv.axon_hooks"] = mod


def _prepare_in_maps(x, w_qkv, b_qkv, w_proj, b_proj):
    x = np.asarray(x, dtype=np.float32)
    b, c, h, w = x.shape
    assert (b, c, h, w) == (B, C, 32, 32)

    wext, bqkc = _pack_weights(w_qkv, b_qkv, w_proj, b_proj)
    wext_bf = _bf16(wext)
    bqkc = np.ascontiguousarray(bqkc, dtype=np.float32)

    in_maps = []
    for core in range(N_CORES):
        xm = np.ascontiguousarray(x[core].reshape(C, L))
        xw = np.empty((128, XW_W), dtype=wext_bf.dtype)
        xw[:, 0:1024] = _bf16(xm[0:128])
        xw[:, 1024:2048] = _bf16(xm[128:256])
        xw[:, 2048:XW_W] = wext_bf
        m = {"xw": xw, "bqkc": bqkc}
        in_maps.append(m)
    return in_maps


def kernel(x, w_qkv, b_qkv, w_proj, b_proj, _trace=False, _trace_kwargs=None):
    if _trace:
        _install_ntff_hook_module()
    from concourse.bass_utils import run_bass_kernel_spmd

    in_maps = _prepare_in_maps(x, w_qkv, b_qkv, w_proj, b_proj)
    nc = _get_nc()

    res = run_bass_kernel_spmd(
        nc,
        in_maps,
        list(range(N_CORES)),
        trace=_trace,
        **(_trace_kwargs or {}),
    )
    out = np.stack([res.results[core]["out"] for core in range(N_CORES)])
    if _trace:
        _CACHE["last_result"] = res
    return out.reshape(B, C, 32, 32)


# revision 25
# speedup vs baseline: 1.0268x; 1.0046x over previous
"""MHSA block (b=8, c=256, h=w=32, nh=8) on 8 Trainium2 cores.

Sharding: pure data parallel -- one batch element per NeuronCore, no
collectives.  Per-core algorithm (X = x[b] as (C=256, L=1024)):

  QK    = Wqk @ X  (+bqk via DVE tensor_scalar add)               (512, L)
  V^T   = X^T @ WvT                                               (L, 256)
  S^T_h = K_h^T Q_h   4 heads concurrently via 4x row tiling
                      (tile_position=(32m,0), K=32 each)          (128, 1024)/jc
  P^T   = exp(scale*S^T)   (ScalarE -- the ~64us/core floor)
  O_h   = V_h^T.T @ P^T_h  4 heads concurrently via 4x col tiling
                           (tile_position=(0,32m), M=32 each;
                            PSUM accumulation over j)             (128, 512)/quad
  l_h   = ones^T @ P^T_h   4 heads col-tiled, lhsT=[1|0..0](128,32)
                           so every PSUM partition is written     row 32m = l
  O_norm = O * bcast(1/l)  (reciprocal_approx_fast + E-matmul broadcast)
  out    = (x + bproj + Wproj @ bv) + Wproj @ O_norm  (biases folded into
                                                       the fp32 residual)

All matmul operands are bf16 (PSUM accumulates fp32).  Schedule: S^T/exp/PV
run as a software pipeline (PV lags 2 slots); normalization/projection
chains are deferred into the next quad's slack.  PSUM budget (8 banks):
2x S^T (128,1024) double-buffered = 4, PV accum + denom accum x 2 quad
parities = 4.  QK / V^T / broadcast / proj borrow those slots.
"""

import sys
import os

sys.path.insert(0, "/opt/trn_rl_repo")

from contextlib import ExitStack

import numpy as np

NH, DH, C, L = 8, 32, 256, 1024
B = 8
SCALE = DH ** -0.5
N_CORES = 8
XW_W = 4384


_CACHE = {}


def _build_nc():
    import concourse.tile as tile
    from concourse import bacc, mybir

    f32 = mybir.dt.float32
    bf16 = mybir.dt.bfloat16
    Exp = mybir.ActivationFunctionType.Exp
    Identity = mybir.ActivationFunctionType.Identity

    nc = bacc.Bacc("TRN2", target_bir_lowering=False, debug=False)

    xw_d = nc.dram_tensor("xw", [128, XW_W], bf16, kind="ExternalInput").ap()
    bqkc_d = nc.dram_tensor("bqkc", [128, 6], f32, kind="ExternalInput").ap()
    out_d = nc.dram_tensor("out", [C, L], f32, kind="ExternalOutput").ap()

    with tile.TileContext(nc) as tc, ExitStack() as ctx:
        persist = ctx.enter_context(tc.tile_pool(name="persist", bufs=1))
        ptpool = ctx.enter_context(tc.tile_pool(name="pt", bufs=12))
        onpool = ctx.enter_context(tc.tile_pool(name="on", bufs=2))
        smallp = ctx.enter_context(tc.tile_pool(name="small", bufs=2))
        stps = ctx.enter_context(tc.tile_pool(name="stps", bufs=3, space="PSUM"))
        pvps = ctx.enter_context(tc.tile_pool(name="pvps", bufs=1, space="PSUM"))

        xw = persist.tile([128, XW_W], bf16, tag="xw", name="xw")
        # a-half x columns + wqk first: they alone gate the first QK matmuls
        nc.sync.dma_start(xw[:, 0:512], xw_d[:, 0:512])
        nc.sync.dma_start(xw[:, 1024:1536], xw_d[:, 1024:1536])
        nc.sync.dma_start(xw[:, 2048:2560], xw_d[:, 2048:2560])
        nc.sync.dma_start(xw[:, 2560:3072], xw_d[:, 2560:3072])
        nc.sync.dma_start(xw[:, 3072:3584], xw_d[:, 3072:3584])
        nc.sync.dma_start(xw[:, 512:1024], xw_d[:, 512:1024])
        nc.sync.dma_start(xw[:, 1536:2048], xw_d[:, 1536:2048])
        x_sb = [xw[:, 0:1024], xw[:, 1024:2048]]
        wqk_sb_w = [xw[:, 2048:2560], xw[:, 2560:3072]]
        wv_sb = [xw[:, 3072:3328], xw[:, 3328:3584]]
        wp_sb = [xw[:, 3584:3840], xw[:, 3840:4096]]
        ep_sb = xw[:, 4096:4224]
        onz_sb = xw[:, 4224:4256]
        zl_sb = xw[:, 4256:4384]  # zero lhsT for PSUM pre-clear matmuls

        bqkc_sb = persist.tile([128, 6], f32, tag="bqkc", name="bqkc")
        nc.sync.dma_start(bqkc_sb[:], bqkc_d[:])

        # warm the ACT exp table while the DMAs/QKV phase run
        warm = persist.tile([1, 8], f32, tag="warm", name="warm")
        nc.gpsimd.memset(warm[:], 0.0)
        nc.scalar.activation(warm[:], warm[:], Exp)

        # ---- QK gemm:  QK(512, L) = WqkT.T @ X; bqk added on the DVE copy.
        # Emitted in (128,512) halves on the pv/dn PSUM banks so the S^T
        # double-buffer is never disturbed and the first S^T only waits on
        # the minimal 4 matmuls. ----
        qk_sb = [persist.tile([128, L], bf16, tag=f"qk{mt}", name=f"qk{mt}")
                 for mt in range(4)]

        def qk_full(mt):
            pst = stps.tile([128, L], f32, tag="st", name=f"qkfs{mt}")
            for kt in range(2):
                for half in range(2):
                    nc.tensor.matmul(
                        pst[:, half * 512:(half + 1) * 512],
                        lhsT=wqk_sb_w[kt][:, mt * 128:(mt + 1) * 128],
                        rhs=x_sb[kt][:, half * 512:(half + 1) * 512],
                        start=(kt == 0),
                        stop=(kt == 1),
                    )
            nc.vector.tensor_scalar_add(qk_sb[mt][:], pst[:], bqkc_sb[:, mt:mt + 1])

        def qk_half(mt, half, tag, use_act=False):
            pst = stps.tile([128, L], f32, tag="st", name=f"qkps{mt}{half}")
            ps = pst[:, 0:512]
            for kt in range(2):
                nc.tensor.matmul(
                    ps,
                    lhsT=wqk_sb_w[kt][:, mt * 128:(mt + 1) * 128],
                    rhs=x_sb[kt][:, half * 512:(half + 1) * 512],
                    start=(kt == 0),
                    stop=(kt == 1),
                )
            o = qk_sb[mt][:, half * 512:(half + 1) * 512]
            if use_act:
                # ScalarE is idle before the first exp; keep the first S^T's
                # bias adds off the (slow-to-wake) DVE path
                nc.scalar.activation(o, ps, Identity, bias=bqkc_sb[:, mt:mt + 1])
            else:
                nc.vector.tensor_scalar_add(o, ps, bqkc_sb[:, mt:mt + 1])

        # ---- V^T gemm: VT(L, 256) = X.T @ WvT  (no bias; folded in residual) ----
        vt_sb = [None] * 8

        def vt_chunk(jt):
            pst = stps.tile([128, L], f32, tag="st", name="vtps")
            ps = pst
            for kt in range(2):
                nc.tensor.matmul(
                    ps[:, 0:256],
                    lhsT=x_sb[kt][:, jt * 128:(jt + 1) * 128],
                    rhs=wv_sb[kt],
                    start=(kt == 0),
                    stop=(kt == 1),
                )
            vt = persist.tile([128, 256], bf16, tag=f"vt{jt}", name=f"vt{jt}")
            nc.vector.tensor_copy(vt[:], ps[:, 0:256])
            vt_sb[jt] = vt

        # PE warm-up: ~3.5us of junk matmuls during the input DMA window so
        # HAM grants 2.4 GHz before the first QK matmul
        scratch = persist.tile([128, 512], bf16, tag="scratch", name="scratch")
        nc.gpsimd.memset(scratch[:], 0.0)
        wps = pvps.tile([128, 512], f32, tag="pv", name="warmps")
        for i in range(8):
            nc.tensor.matmul(wps[:], lhsT=scratch[:, 0:128], rhs=scratch[:],
                             start=(i == 0), stop=(i == 7))
        nc.vector.tensor_copy(scratch[:, 0:128], wps[:, 0:128])

        qk_half(0, 0, "st", use_act=True)
        qk_half(2, 0, "st", use_act=True)
        nc.sync.dma_start(xw[:, 3584:XW_W], xw_d[:, 3584:XW_W])

        # residual xf = bf16(x) + (bproj + Wproj@bv) computed on-chip; saves
        # the 1MB fp32 DMA that would otherwise fight x/w for startup HBM BW
        xf_sb = [persist.tile([128, L], f32, tag=f"xf{t}", name=f"xf{t}")
                 for t in range(2)]

        def xf_make():
            for t in range(2):
                nc.vector.tensor_scalar_add(xf_sb[t][:], x_sb[t], bqkc_sb[:, 4 + t:5 + t])

        acc = [persist.tile([128, L], f32, tag=f"acc{t}", name=f"acc{t}") for t in range(2)]

        deferred = []
        on_holder = {}

        def make_chain_a1(tg, ih, qi, pv, dn):
            def chain_a1():
                # rows 32m of dn hold l_h; other rows are exact zeros, so the
                # E' broadcast matmul (E'[32k, 32k+d] = 1) replicates l_h to
                # its head's 32 rows without picking up garbage.  a1 fully
                # evacuates BOTH accumulator banks to SBUF so the next quad's
                # accumulators can claim them after two DVE copies instead of
                # after the whole recip/mul chain.
                df = smallp.tile([128, 512], bf16, tag="df", name="df")
                nc.vector.tensor_copy(df[:], dn[:])
                if qi < 3:
                    pvs = smallp.tile([128, 512], bf16, tag="pvs", name="pvs")
                    nc.vector.tensor_copy(pvs[:], pv[:])
                else:
                    pvs = pv  # no next quad waits on the bank; mul reads PSUM
                on_holder[("df", qi)] = (df, pvs)

            return chain_a1

        def make_chain_a2(tg, ih, qi, pv, dn):
            parts = ((0, 256), (256, 256)) if qi == 3 else ((0, 512),)

            def chain_a2():
                df, pvs = on_holder.pop(("df", qi))
                on = onpool.tile([128, 512], bf16, tag="on", name="on")
                lbt = stps.tile([128, L], f32, tag="st", name="lb")
                lb = lbt[:, 0:512]
                for o, w in parts:
                    nc.tensor.matmul(lb[:, o:o + w], lhsT=ep_sb, rhs=df[:, o:o + w],
                                     start=True, stop=True)
                    rps = smallp.tile([128, 512], f32, tag="rps", name="rps")
                    nc.vector.reciprocal_approx_fast(rps[:, 0:w], lb[:, o:o + w])
                    nc.vector.tensor_mul(on[:, o:o + w], pvs[:, o:o + w], rps[:, 0:w])
                on_holder[qi] = on

            return chain_a2

        def make_chain_b(tg, ih, qi):
            cols = slice(ih * 512, (ih + 1) * 512)

            def chain_b():
                on = on_holder.pop(qi)
                pj = stps.tile([128, L], f32, tag="st", name="pj")
                for mt in range(2):
                    nc.tensor.matmul(
                        pj[:, mt * 512:(mt + 1) * 512],
                        lhsT=wp_sb[tg][:, mt * 128:(mt + 1) * 128],
                        rhs=on[:],
                        start=True,
                        stop=True,
                    )
                for mt in range(2):
                    pjv = pj[:, mt * 512:(mt + 1) * 512]
                    if tg == 0:
                        nc.vector.tensor_add(acc[mt][:, cols], xf_sb[mt][:, cols], pjv)
                    else:
                        nc.vector.tensor_add(acc[mt][:, cols], acc[mt][:, cols], pjv)
                        nc.sync.dma_start(out_d[mt * 128:(mt + 1) * 128, cols], acc[mt][:, cols])

            return chain_b

        def make_quad(qi, tg, ih):
            cols = slice(ih * 512, (ih + 1) * 512)
            state = {}

            def st_fn(jc):
                for h in iter_hooks.pop((tg, ih, jc), []):
                    h()
                qt = qk_sb[tg]
                kt_ = qk_sb[2 + tg]
                sts = [
                    stps.tile([128, L], f32, tag="st", name="stA"),
                    stps.tile([128, L], f32, tag="st", name="stB"),
                ]
                for m in range(4):
                    o = 32 * m
                    nc.tensor.matmul(
                        sts[m // 2][:, (m % 2) * 512:(m % 2) * 512 + 512],
                        lhsT=kt_[o:o + 32, jc * 128:(jc + 1) * 128],
                        rhs=qt[o:o + 32, cols],
                        start=True,
                        stop=True,
                        tile_position=(o, 0),
                    )
                pts = []
                for half in range(2):
                    pt = ptpool.tile([128, L], bf16, tag="pt", name="pt")
                    nc.scalar.activation(pt[:], sts[half][:], Exp, scale=SCALE)
                    pts.append(pt)
                state[jc] = pts

            def pv_fn(jc):
                if jc == 0:
                    # Pre-zero both accumulators with one full-array matmul
                    # each: concurrent col-tiled matmuls must not carry
                    # start=True (the bank-wide has_written clear races
                    # against sibling tiles writing the same bank).
                    state["pv"] = pvps.tile([128, 512], f32, tag="pv", name="pvacc")
                    state["dn"] = pvps.tile([128, 512], f32, tag="dn", name="dnacc")
                    for z in ("pv", "dn"):
                        nc.tensor.matmul(
                            state[z][:],
                            lhsT=zl_sb,
                            rhs=x_sb[0][:, 0:512],
                            start=True,
                            stop=True,
                        )
                pts = state.pop(jc)
                pv, dn = state["pv"], state["dn"]
                for m in range(4):
                    nc.tensor.matmul(
                        pv[32 * m:32 * m + 32, :],
                        lhsT=vt_sb[jc][:, (4 * tg + m) * 32:(4 * tg + m) * 32 + 32],
                        rhs=pts[m // 2][:, (m % 2) * 512:(m % 2) * 512 + 512],
                        start=False,
                        stop=(jc == 7),
                        tile_position=(0, 32 * m),
                    )
                for m in range(4):
                    nc.tensor.matmul(
                        dn[32 * m:32 * m + 32, :],
                        lhsT=onz_sb,
                        rhs=pts[m // 2][:, (m % 2) * 512:(m % 2) * 512 + 512],
                        start=False,
                        stop=(jc == 7),
                        tile_position=(0, 32 * m),
                    )
                if jc == 7:
                    deferred.append(make_chain_a1(tg, ih, qi, pv, dn))
                    deferred.append(make_chain_a2(tg, ih, qi, pv, dn))
                    deferred.append(make_chain_b(tg, ih, qi))

            return st_fn, pv_fn

        quads = [(0, 0), (0, 1), (1, 0), (1, 1)]
        iter_hooks = {
            (0, 0, 1): [lambda: qk_half(2, 1, "st"),
                        lambda: [vt_chunk(j) for j in range(2)]],
            (0, 0, 3): [xf_make, lambda: [vt_chunk(j) for j in range(2, 4)]],
            (0, 0, 4): [lambda: [vt_chunk(j) for j in range(4, 6)]],
            (0, 0, 5): [lambda: qk_half(0, 1, "st")],
            (0, 0, 6): [lambda: [vt_chunk(j) for j in range(6, 8)]],
            (0, 1, 1): [lambda: qk_full(1)],
            (0, 1, 6): [lambda: qk_full(3)],
        }
        # Explicit slot schedule.  Slot s = 8q + r emits S^T(q, r); the PV
        # groups lag 2-3 slots; the norm/proj chain of quad q-1 is split
        # across r = 2 (DVE copy), r = 3 (broadcast+recip+mul, emitted just
        # before quad q's accumulator allocs so the pv/dn banks hand over
        # without a cycle), and r = 5 (projection).
        fns = [make_quad(qi, tg, ih) for qi, (tg, ih) in enumerate(quads)]
        pv_slots = {}
        for q in range(4):
            for j in range(8):
                pv_slots.setdefault(8 * q + j + 2, []).append((q, j))

        def emit_pvs(s):
            for pq, pj in pv_slots.get(s, []):
                fns[pq][1](pj)

        for s in range(32):
            q, r = divmod(s, 8)
            fns[q][0](r)
            # chain a1 frees the pv/dn banks this slot's pv(q,0) allocs claim
            if r in (2, 3, 5) and deferred:
                deferred.pop(0)()
            emit_pvs(s)
        for s in (32, 33):
            emit_pvs(s)
        while deferred:
            deferred.pop(0)()

    nc.compile()
    return nc


def _get_nc():
    if "nc" not in _CACHE:
        _CACHE["nc"] = _build_nc()
    return _CACHE["nc"]


def _pack_weights(w_qkv, b_qkv, w_proj, b_proj):
    w_qkv = np.asarray(w_qkv, dtype=np.float32)
    b_qkv = np.asarray(b_qkv, dtype=np.float32)
    w_proj = np.asarray(w_proj, dtype=np.float32)
    b_proj = np.asarray(b_proj, dtype=np.float32)

    wqkT = np.ascontiguousarray(w_qkv[:512].T)                  # (256, 512)
    resid_bias = b_proj + w_proj @ b_qkv[512:768]
    bqkc = np.zeros((128, 6), dtype=np.float32)
    bqkc[:, 0:4] = b_qkv[:512].reshape(4, 128).T
    bqkc[:, 4] = resid_bias[0:128]
    bqkc[:, 5] = resid_bias[128:256]
    wvT = np.ascontiguousarray(w_qkv[512:768].T)                # (256, 256)
    wpT = np.ascontiguousarray(w_proj.T)                        # (256, 256)

    wext = np.zeros((128, XW_W - 2048), dtype=np.float32)
    wext[:, 0:512] = wqkT[0:128]
    wext[:, 512:1024] = wqkT[128:256]
    wext[:, 1024:1280] = wvT[0:128]
    wext[:, 1280:1536] = wvT[128:256]
    wext[:, 1536:1792] = wpT[0:128]
    wext[:, 1792:2048] = wpT[128:256]
    # E' broadcast matrix (128,128): E'[32k, 32k+d] = 1 replicates the
    # denominator at row 32k to rows 32k..32k+31 of its head
    for k in range(4):
        wext[32 * k, 2048 + 32 * k:2048 + 32 * k + 32] = 1.0
    # ones|zeros (128,32) lhsT for the denominator matmuls
    wext[:, 2176] = 1.0
    # the residual carries x + bproj + Wproj @ bv (the V-bias contribution:
    # O_norm = O/l + bv, and Wproj @ bv is column-constant); it is built
    # on-chip from bf16(x) + bqkc cols 4:6
    return wext, bqkc


def _bf16(a):
    import ml_dtypes

    return np.asarray(a).astype(ml_dtypes.bfloat16)


def _install_ntff_hook_module():
    """bass_utils wants antenv.axon_hooks for trace=True under axon; this
    image's antenv lacks it.  Inject an equivalent module into sys.modules."""
    if "antenv.axon_hooks" in sys.modules:
        return
    try:
        import antenv.axon_hooks  # noqa: F401

        return
    except ImportError:
        pass
    import contextlib
    import ctypes
    import types

    mod = types.ModuleType("antenv.axon_hooks")
    state = {"hook": None, "inited": False}

    def _default_hook():
        so_path = "/opt/axon/libaxon_pjrt.so"
        if not os.path.exists(so_path):
            return None
        lib = ctypes.CDLL(so_path)
        if not hasattr(lib, "axon_start_nrt_profile"):
            return None
        lib.axon_start_nrt_profile.argtypes = [
            ctypes.POINTER(ctypes.c_int64),
            ctypes.c_size_t,
        ]
        lib.axon_start_nrt_profile.restype = ctypes.c_int64
        lib.axon_stop_nrt_profile.argtypes = [ctypes.c_char_p]
        lib.axon_stop_nrt_profile.restype = ctypes.c_int64

        @contextlib.contextmanager
        def _hook(output_dir, device_ids):
            import jax

            jax.devices()
            if device_ids:
                ids = (ctypes.c_int64 * len(device_ids))(*device_ids)
                rc = lib.axon_start_nrt_profile(ids, len(device_ids))
            else:
                rc = lib.axon_start_nrt_profile(None, 0)
            if rc != 0:
                raise RuntimeError(f"axon_start_nrt_profile rc={rc}")
            try:
                yield
            finally:
                n = lib.axon_stop_nrt_profile(str(output_dir).encode())
                if n < 0:
                    raise RuntimeError(f"axon_stop_nrt_profile rc={n}")
                print(f"profile: {n} file(s) written to {output_dir}")

        return _hook

    def set_axon_ntff_profile_hook(hook):
        state["hook"] = hook
        state["inited"] = True

    def get_axon_ntff_profile_hook():
        if not state["inited"]:
            state["hook"] = _default_hook()
            state["inited"] = True
        return state["hook"]

    mod.set_axon_ntff_profile_hook = set_axon_ntff_profile_hook
    mod.get_axon_ntff_profile_hook = get_axon_ntff_profile_hook
    sys.modules["antenv.axon_hooks"] = mod


def _prepare_in_maps(x, w_qkv, b_qkv, w_proj, b_proj):
    x = np.asarray(x, dtype=np.float32)
    b, c, h, w = x.shape
    assert (b, c, h, w) == (B, C, 32, 32)

    wext, bqkc = _pack_weights(w_qkv, b_qkv, w_proj, b_proj)
    wext_bf = _bf16(wext)
    bqkc = np.ascontiguousarray(bqkc, dtype=np.float32)

    in_maps = []
    for core in range(N_CORES):
        xm = np.ascontiguousarray(x[core].reshape(C, L))
        xw = np.empty((128, XW_W), dtype=wext_bf.dtype)
        xw[:, 0:1024] = _bf16(xm[0:128])
        xw[:, 1024:2048] = _bf16(xm[128:256])
        xw[:, 2048:XW_W] = wext_bf
        m = {"xw": xw, "bqkc": bqkc}
        in_maps.append(m)
    return in_maps


def kernel(x, w_qkv, b_qkv, w_proj, b_proj, _trace=False, _trace_kwargs=None):
    if _trace:
        _install_ntff_hook_module()
    from concourse.bass_utils import run_bass_kernel_spmd

    in_maps = _prepare_in_maps(x, w_qkv, b_qkv, w_proj, b_proj)
    nc = _get_nc()

    res = run_bass_kernel_spmd(
        nc,
        in_maps,
        list(range(N_CORES)),
        trace=_trace,
        **(_trace_kwargs or {}),
    )
    out = np.stack([res.results[core]["out"] for core in range(N_CORES)])
    if _trace:
        _CACHE["last_result"] = res
    return out.reshape(B, C, 32, 32)
